# revision 36
# baseline (speedup 1.0000x reference)
"""BandSplit kernel for Trainium2 (8 NeuronCores, SPMD data-parallel).

Math: the (deterministic) melbank partitions the 1025 STFT bins into 257
contiguous segments (widths 1/4/8/8/1), all mel weights are 1.0, so

    out[b,c,t,k,o] = sum_{f in seg(k)} sum_i x[b,c,t,f,i]*pre_w[i,f,o] + pre_b[k,o]

Sharding: data-parallel over the 8 (b,c) pairs, one per core.
Per core: 256 tokens; out (256, 257, 128) = 8.4M elems (memory bound).

Device strategy: per-band segment matmuls on the PE, packed 2-4 bands per
matmul as a block-diagonal rhs (K = sum 2w + 1 bias ones-row, N = nb*128),
with lhsT = transposed token-major x slices at 32-aligned partition offsets
so 2-4 matmuls share a 128-row column group (PE row-tiling). All DRAM I/O
is fp16 (~1e-3 rel err, inside the 2e-2 gate): inputs are packed to fp16 on
host, matmuls run fp16 (fp32 PSUM accumulate), PSUM -> SBUF copies cast to
fp16 staging tiles. Inputs live padded in DRAM but are loaded with
partition-sliced DMAs that skip the alignment padding (~2.5 MB reads).
PSUM -> SBUF copies alternate VectorE/ScalarE into merged staging tiles
(4-9 matmuls each); each staging tile is written to DRAM with a single
fully-contiguous block DMA (host reassembles the final fp32 layout).
Output DMAs alternate the two HWDGE rings (sync / scalar).
"""

import numpy as np

import concourse.bacc as bacc
import concourse.mybir as mybir
from concourse.tile import TileContext
from concourse.bass_utils import run_bass_kernel_spmd

# ---------------------------------------------------------------- structure

B, C, T, NF, IN_CH = 4, 2, 256, 1025, 2
N_BANDS, OUT_CH = 257, 128
N_CORES = 8
TOK = 256           # tokens per core (= T; one (b,c) pair per core)
HALVES = 2          # 128-token tiles


def _segments():
    segs = []
    for k in range(N_BANDS):
        if k < 128:
            segs.append((k, 1))
        elif k < 160:
            segs.append((128 + 4 * (k - 128), 4))
        elif k < 192:
            segs.append((256 + 8 * (k - 160), 8))
        elif k < 256:
            segs.append((512 + 8 * (k - 192), 8))
        else:
            segs.append((1024, 1))
    return segs


SEGS = _segments()


def _build_plan():
    """Matmul descriptors: bands (2-4 contiguous), x-group g, 32-aligned
    partition offset, K rows (sum 2w + ones bias row), N out cols, W region
    col start."""
    plan = []
    for a in range(33):  # class A: width-1 bands 0..127 (K=9) + band 256 (K=3)
        bands = [256] if a == 32 else list(range(4 * a, 4 * a + 4))
        plan.append(dict(
            bands=bands, g=a // 4, off=32 * (a % 4),
            K=sum(2 * SEGS[k][1] for k in bands) + 1,
            N=128 * len(bands),
            wcol=512 * (a // 4) if a < 32 else 4096,
        ))
    for b in range(8):   # class B: width-4 bands 128..159 (K=33)
        bands = list(range(128 + 4 * b, 128 + 4 * b + 4))
        plan.append(dict(
            bands=bands, g=9 + b // 2, off=64 * (b % 2),
            K=33, N=512,
            wcol=4224 + 512 * (b // 2),
        ))
    for c in range(48):  # class C: width-8 bands 160..255 (K=33)
        bands = [160 + 2 * c, 160 + 2 * c + 1]
        plan.append(dict(
            bands=bands, g=13 + c // 2, off=64 * (c % 2),
            K=33, N=256,
            wcol=4224 + 2048 + 256 * (c // 2),
        ))
    return plan


PLAN = _build_plan()
NG = 37                      # x column groups
XCOLS = NG * TOK             # 9472
WCOLS = 4224 + 2048 + 6144   # 12416

# merge groups: PLAN mms staged together and written with one contiguous
# block DMA (order is free — host reassembles). Consecutive mms ALTERNATE
# PE row groups (off 0/32 or 0/64) so LDWEIGHTS pulls ahead of in-flight
# MATMULs and bank-disjoint pairs run concurrently in the PE sub-arrays.
# ~1.0 MB fp16 output DMAs.
_MERGE = (
    # Block 1: 2-way warm-up (only the first two x/w loads needed).
    [[0, 1, 4, 5, 8, 9, 12, 13]]                   # A g0-3  @ off 0/32
    # Blocks 2-3: 4-way row groups — A off0/32 (g4-7) interleaved with
    # A off64/96 (g0-3): 4 concurrent MMs fill 2 psum tiles per stream.
    + [[16, 17, 2, 3, 20, 21, 6, 7]]
    + [[24, 25, 10, 11, 28, 29, 14, 15]]
    # Blocks 4-5: A g4-7 @ off64/96 interleaved with B @ off0/64 (+band 256)
    + [[18, 33, 19, 34, 22, 35, 23, 36, 32]]
    + [[26, 37, 27, 38, 30, 39, 31, 40]]
    # C: 4 mms (N=256) share a 2-bank psum tile; order [c, c+2, c+1, c+3]
    # puts same-bank pairs in the SAME row group (PE serializes them; no
    # concurrent same-bank PSUM writes) while banks still alternate groups.
    + [[41 + c + d for c in range(0, 16, 4) for d in (0, 2, 1, 3)]]
    + [[41 + c + d for c in range(16, 32, 4) for d in (0, 2, 1, 3)]]
    + [[41 + c + d for c in range(32, 48, 4) for d in (0, 2, 1, 3)]]
)
MAXMERGE = max(sum(PLAN[i]["N"] for i in m) for m in _MERGE)  # 4224

# (h, merge) -> flat output offset; total flat size. Half 1 runs the
# merges in reverse. The first/last two scheduled merges are split into
# ~2048-col chunks so the output DMA ring starts earlier / drains sooner.
def _split_merge(m, target=2048):
    parts, cur, n = [], [], 0
    for i in m:
        cur.append(i)
        n += PLAN[i]["N"]
        if n >= target:
            parts.append(cur)
            cur, n = [], 0
    if cur:
        parts.append(cur)
    return parts


_SCHED = [(0, m) for m in _MERGE] + [(1, m) for m in _MERGE[::-1]]
_OBLOCKS = []
_o = 0
for _h, _m in _SCHED:
    ntot = sum(PLAN[i]["N"] for i in _m)
    _OBLOCKS.append((_h, _m, _o, ntot))
    _o += 128 * ntot
OELEMS = _o  # == TOK * N_BANDS * OUT_CH

# partition-sliced load regions (row_lo, row_hi, col_lo, col_hi), skipping
# the 32-alignment padding rows. Loads are emitted lazily, two blocks
# ahead of first use (x on the scalar ring, w on the sync ring).
_XLOADS = {
    "x0a": (0, 9, 0, 1024), "x32a": (32, 41, 0, 1024),
    "x64a": (64, 73, 0, 1024), "x96a": (96, 105, 0, 1024),
    "x0b": (0, 9, 1024, 2304), "x32b": (32, 41, 1024, 2304),
    "x64b": (64, 73, 1024, 2304), "x96b": (96, 105, 1024, 2304),
    "xbc0": (0, 33, 2304, XCOLS), "xbc64": (64, 97, 2304, XCOLS),
}
_WLOADS = {
    "w0a": (0, 9, 0, 2048), "w32a": (32, 41, 0, 2048),
    "w64a": (64, 73, 0, 2048), "w96a": (96, 105, 0, 2048),
    "w0b": (0, 9, 2048, 4096), "w32b": (32, 41, 2048, 4096),
    "w64b": (64, 73, 2048, 4096), "w96b": (96, 105, 2048, 4096),
    "wb256": (0, 3, 4096, 4224),
    "wbc0a": (0, 33, 4224, 6272), "wbc64a": (64, 97, 4224, 6272),
    "wbc0b": (0, 33, 6272, WCOLS), "wbc64b": (64, 97, 6272, WCOLS),
}
# loads first needed by each half-0 block (see _MERGE order)
_BLOCK_LOADS = {
    0: (["x0a", "x32a"], ["w0a", "w32a"]),
    1: (["x0b", "x32b", "x64a", "x96a"],
        ["w0b", "w32b", "w64a", "w96a"]),
    3: (["x64b", "x96b", "xbc0", "xbc64"],
        ["w64b", "w96b", "wbc0a", "wbc64a", "wb256"]),
    5: ([], ["wbc0b", "wbc64b"]),
}


def _xmm_index():
    """Fancy-index arrays to build x_mm from xt (2050, TOK)."""
    src, dstg, dstr, og, orow = [], [], [], [], []
    for mm in PLAN:
        r = 0
        for k in mm["bands"]:
            f0, w = SEGS[k]
            for l in range(w):
                for i in range(IN_CH):
                    src.append((f0 + l) * 2 + i)
                    dstg.append(mm["g"])
                    dstr.append(mm["off"] + r)
                    r += 1
        og.append(mm["g"])
        orow.append(mm["off"] + r)
    return (np.array(src), np.array(dstg), np.array(dstr),
            np.array(og), np.array(orow))


_XSRC, _XDG, _XDR, _XOG, _XOR = _xmm_index()

# ---------------------------------------------------------------- host prep


def _build_wmm(pre_w, pre_b):
    """(128, WCOLS) fp16: per-mm block-diagonal weights + bias ones-row."""
    wmm = np.zeros((128, WCOLS), dtype=np.float32)
    for mm in PLAN:
        off, wc = mm["off"], mm["wcol"]
        r = 0
        for j, k in enumerate(mm["bands"]):
            f0, w = SEGS[k]
            cols = slice(wc + 128 * j, wc + 128 * (j + 1))
            for l in range(w):
                for i in range(IN_CH):
                    wmm[off + r, cols] = pre_w[i, f0 + l, :]
                    r += 1
            wmm[off + mm["K"] - 1, cols] = pre_b[k, :]
    return wmm.astype(np.float16)


def _build_xmm(x_core):
    """x_core (TOK, NF, IN_CH) -> (128, XCOLS) fp16 packed lhsT layout."""
    xt = np.ascontiguousarray(x_core.reshape(TOK, NF * IN_CH).T)  # (2050, TOK)
    xmm = np.zeros((NG, 128, TOK), dtype=np.float16)
    xmm[_XDG, _XDR, :] = xt[_XSRC, :].astype(np.float16)
    xmm[_XOG, _XOR, :] = 1.0
    return np.ascontiguousarray(xmm.transpose(1, 0, 2)).reshape(128, XCOLS)


def _assemble(out_flat):
    """flat device output -> (TOK, N_BANDS, OUT_CH)."""
    oc = np.empty((TOK, N_BANDS, OUT_CH), dtype=np.float32)
    for h, merge, o, ntot in _OBLOCKS:
        blk = out_flat[o:o + 128 * ntot].reshape(128, ntot)
        c = 0
        for i in merge:
            mm = PLAN[i]
            nb = len(mm["bands"])
            k0 = mm["bands"][0]
            oc[h * 128:(h + 1) * 128, k0:k0 + nb, :] = (
                blk[:, c:c + mm["N"]].reshape(128, nb, OUT_CH))
            c += mm["N"]
    return oc


# ---------------------------------------------------------------- device

_PROGRAM = None


def _build_program():
    global _PROGRAM
    if _PROGRAM is not None:
        return _PROGRAM

    nc = bacc.Bacc("TRN2", target_bir_lowering=False)
    f32 = mybir.dt.float32
    f16 = mybir.dt.float16
    xin = nc.dram_tensor("xmm", [128, XCOLS], f16, kind="ExternalInput")
    win = nc.dram_tensor("wmm", [128, WCOLS], f16, kind="ExternalInput")
    out = nc.dram_tensor("out", [OELEMS], f16, kind="ExternalOutput")

    with TileContext(nc) as tc:
        with (
            tc.tile_pool(name="xw", bufs=1) as xw_pool,
            tc.tile_pool(name="stage", bufs=10) as stage_pool,
            tc.tile_pool(name="psum", bufs=4, space="PSUM") as psum_pool,
        ):
            x_sb = xw_pool.tile([128, XCOLS], f16, tag="x")
            w_sb = xw_pool.tile([128, WCOLS], f16, tag="w")

            # prewarm the scalar ACT table (first ACTIVATE pays ~2.7us
            # table load otherwise) with a dependency-free dummy copy.
            warm = stage_pool.tile([128, 2], f16, tag="warm")
            nc.vector.memset(warm[:, 0:1], 0.0)
            nc.scalar.copy(warm[:, 1:2], warm[:, 0:1])

            # greedy engine balance for PSUM->SBUF copies (ns models);
            # scalar pays for each input-load / output-DMA it issues.
            ebusy = {"v": 0.0, "s": 0.0}

            def emit_loads(bj):
                for name in _BLOCK_LOADS.get(bj, ([], []))[0]:
                    r0, r1, c0, c1 = _XLOADS[name]
                    nc.scalar.dma_start(out=x_sb[r0:r1, c0:c1],
                                        in_=xin.ap()[r0:r1, c0:c1])
                    ebusy["s"] += 800.0
                for name in _BLOCK_LOADS.get(bj, ([], []))[1]:
                    r0, r1, c0, c1 = _WLOADS[name]
                    nc.sync.dma_start(out=w_sb[r0:r1, c0:c1],
                                      in_=win.ap()[r0:r1, c0:c1])

            def copy(dst, src, n):
                tv = ebusy["v"] + (120 + n) / 0.96
                ts = ebusy["s"] + (172 + n) / 1.2
                if tv <= ts:
                    ebusy["v"] = tv
                    nc.vector.tensor_copy(dst, src)
                else:
                    ebusy["s"] = ts
                    nc.scalar.copy(dst, src)

            def emit_dma(eng, o, ntot, sb):
                eng.dma_start(
                    out=out.ap()[o:o + 128 * ntot]
                        .rearrange("(p n) -> p n", n=ntot),
                    in_=sb[:],
                )

            emit_loads(0)
            emit_loads(1)
            delayed = []  # (emit_at_bi, o, ntot, sb) for the scalar ring
            for bi, (h, merge, o, ntot) in enumerate(_OBLOCKS):
                emit_loads(bi + 2)  # issue loads two blocks ahead
                while delayed and delayed[0][0] <= bi:
                    _, po, pn, psb = delayed.pop(0)
                    emit_dma(nc.scalar, po, pn, psb)
                tcol = h * 128
                sb = stage_pool.tile([128, ntot], f16, tag="st")
                c = 0       # staging cols emitted (incl. current psum tile)
                ps, pcol = None, 0
                for i in merge:
                    mm = PLAN[i]
                    off, K, N = mm["off"], mm["K"], mm["N"]
                    if ps is not None and pcol + N > 1024:
                        copy(sb[:, c - pcol:c], ps[:, :pcol], pcol)
                        ps = None
                    if ps is None:
                        ps = psum_pool.tile([128, 1024], f32, tag="ps")
                        pcol = 0
                    gcol = mm["g"] * TOK + tcol
                    nc.tensor.matmul(
                        ps[:, pcol:pcol + N],
                        x_sb[off:off + K, gcol:gcol + 128],
                        w_sb[off:off + K, mm["wcol"]:mm["wcol"] + N],
                        start=True, stop=True,
                        tile_position=(off, 0),
                    )
                    pcol += N
                    c += N
                copy(sb[:, c - pcol:c], ps[:, :pcol], pcol)
                # alternate output rings; the scalar ring's issue is DELAYED
                # two blocks so its staging wait is already satisfied and
                # never head-of-line-blocks the scalar copy queue.
                if bi % 2 == 1 and bi >= 3:
                    delayed.append((bi + 2, o, ntot, sb))
                    ebusy["s"] += 800.0
                else:
                    emit_dma(nc.sync, o, ntot, sb)
            for _, po, pn, psb in delayed:
                emit_dma(nc.scalar, po, pn, psb)

    nc.compile()
    _PROGRAM = nc
    return nc


# ---------------------------------------------------------------- entry

LAST_RESULTS = None  # BassKernelResults of the most recent run (for test.py)


def kernel(x, pre_w, pre_b, _trace=False):
    global LAST_RESULTS
    x = np.asarray(x, dtype=np.float32)
    pre_w = np.asarray(pre_w, dtype=np.float32)
    pre_b = np.asarray(pre_b, dtype=np.float32)
    assert x.shape == (B, C, T, NF, IN_CH), x.shape

    nc = _build_program()
    wmm = _build_wmm(pre_w, pre_b)
    in_maps = []
    for core in range(N_CORES):
        b_, c_ = divmod(core, C)
        in_maps.append({"xmm": _build_xmm(x[b_, c_]), "wmm": wmm})

    res = run_bass_kernel_spmd(
        nc, in_maps, core_ids=list(range(N_CORES)), trace=_trace,
    )
    LAST_RESULTS = res

    out = np.empty((B, C, T, N_BANDS, OUT_CH), dtype=np.float32)
    for core in range(N_CORES):
        b_, c_ = divmod(core, C)
        out[b_, c_] = _assemble(res.results[core]["out"])
    return out

